# revision 72
# baseline (speedup 1.0000x reference)
"""TGCN (dense-graph GRU) Trainium2 kernel, 8-core SPMD, no collectives.

Math (per reference):
  xh_t = relu(x_t @ fc_w + fc_b)                    [N, H]
  S_t  = adj @ xh_t                                 (assoc: adj@(xh@W) = (adj@xh)@W)
  z_t  = sigmoid(S_t @ Mz + h @ Lz_bot + bz)        Mz = Wz @ Lz_top (host-folded)
  r_t  = sigmoid(S_t @ Mr + h @ Lr_bot + br)
  ht_t = tanh   (S_t @ Mh + (h*r) @ Lh_bot + bh)
  h    = z*h + (1-z)*ht = ht + z*(h - ht)

Sharding: row-partition adj across 8 cores (512 nodes each). The GRU cell is
row-local, so each core runs the whole time loop on its shard independently.
x is replicated (each core redundantly computes xh for all nodes — cheaper
than an all-gather through DRAM bounce buffers).

Layout: everything on-chip is feature-major ("transposed"): S_t.T, z.T, h.T
are [64 feat, 512 nodes]. This makes every matmul operand natural-layout
(weights [K, M] as stored, x host-transposed to [T, F, N]) — zero on-chip
transposes. Time steps are processed in pairs so the big adj matmul has
M=128 (full PE array): lhsT = [xh_t | xh_t+1] tiles, rhs = adjT tiles.

Each GRU gate is ONE K=128 matmul: stationary [M*; L*_bot] stacked on the
contraction dim, rhs a [S_t.T; h.T] concat tile whose bottom half IS the
recurrent state (the combine writes h directly into the next concat buffer,
rotating over 6 buffers so gates can lag their pair's S matmul by 2 pairs,
keeping the final S bursts off the chain tail). ACT's ability to
read/write at shifted partition bases glues the [0:64]/[64:128] halves
together; matmul operands at base 64 crash TRN2, so all matmul rhs/lhsT
tiles start at partition 0 or are full 128-partition.

Dtypes: x/fcw/adj fp8e4 (halves the streamed-x DMA and the adjacency
load; the xh matmul computes both steps of a pair in one K=128 matmul
against a block-diagonal fcw with steps stacked on the partition axis).
xh and all matmul stationary operands bf16 -- the S matmul runs
mixed-dtype (bf16 lhsT x fp8 rhs, legal for non-f32 pairs). adj is
host-prescaled by N=2^12 into fp8 range with the inverse folded into the
gate weights (both exact powers of two). 1 cyc/row plain bf16-rate -- NOT
fp8 DoubleRow -- for the big S matmul: the chip's power limiter throttles DoubleRow's 2x MAC density back
to ~1x whenever DVE/ACT run concurrently (measured: 42% -> 8% throttle
residency switching DR -> bf16), so bf16 streams just as fast and leaves
headroom for the chain's ACT/DVE ops. Keep the S burst contiguous per
pair; interleaving it with other matmuls measurably degrades the II. An
AllGather-sharded-xh variant was tried and is SLOWER (collective ring runs
~85 GB/s effective, ~24us per 2MB gather -- communication exceeds the
saved compute). h state bf16; PSUM f32.
Scheduling: xh computation runs TWO pairs ahead of its S burst (a
prologue fills xh for pairs 0-1 while the 4MB adjT load streams in on all
three DMA-capable queues in fine k-ordered chunks), gates lag their pair
by 2, and the final GRU combine writes the f32 output tile directly.
Measured: ~285.6us HW exec (baseline 362us), rel err ~6.6e-3 (tol 2e-2).
Beware device-level run-to-run variance: identical binaries occasionally
measure ~340us (same per-core throttle counters; shared-device state).
"""

import os
import sys

sys.path.insert(0, "/opt/trn_rl_repo")

import numpy as np
import ml_dtypes

T, N, F_IN, H1, F_OUT = 48, 4096, 64, 64, 64
NCORES = 8
NS = N // NCORES          # nodes per core = 512
PAIRS = T // 2            # 24
KT = N // 128             # 32 contraction tiles for the adj matmul

_cache = {}


def _build():
    import concourse.bass as bass
    import concourse.mybir as mybir
    import concourse.tile as tile
    from concourse import bacc

    f32 = mybir.dt.float32
    f32r = mybir.dt.float32r
    bf16 = mybir.dt.bfloat16
    fp8 = mybir.dt.float8e4
    DR = mybir.MatmulPerfMode.DoubleRow
    AF = mybir.ActivationFunctionType

    nc = bacc.Bacc(
        "TRN2",
        target_bir_lowering=False,
        debug=False,
        enable_asserts=False,
        num_devices=NCORES,
    )

    # DRAM parameters (per-core shapes)
    adjT_d = nc.dram_tensor("adjT", [128, KT, NS], fp8, kind="ExternalInput").ap()
    # steps stacked on partitions: one K=128 matmul per node tile computes
    # BOTH steps' xh against a block-diagonal fcw
    xT_d = nc.dram_tensor("xT", [PAIRS, 2 * F_IN, N], fp8, kind="ExternalInput").ap()
    fcw_d = nc.dram_tensor("fcw", [2 * F_IN, 2 * H1], fp8, kind="ExternalInput").ap()
    wzr_d = nc.dram_tensor("wzr", [128, 128], bf16, kind="ExternalInput").ap()
    wh_d = nc.dram_tensor("wh", [128, F_OUT], bf16, kind="ExternalInput").ap()
    bz_d = nc.dram_tensor("bz", [F_OUT, 1], f32, kind="ExternalInput").ap()
    bzn_d = nc.dram_tensor("bzn", [F_OUT, 1], f32, kind="ExternalInput").ap()
    br_d = nc.dram_tensor("br", [F_OUT, 1], f32, kind="ExternalInput").ap()
    bh_d = nc.dram_tensor("bh", [F_OUT, 1], f32, kind="ExternalInput").ap()
    out_d = nc.dram_tensor("out", [F_OUT, NS], f32, kind="ExternalOutput").ap()

    with tile.TileContext(nc) as tc:
        with (
            tc.tile_pool(name="const", bufs=1) as constp,
            tc.tile_pool(name="state", bufs=1) as statep,
            tc.tile_pool(name="xt", bufs=2) as xtp,
            tc.tile_pool(name="xh", bufs=3) as xhp,
            tc.tile_pool(name="gw", bufs=3) as gwp,
            tc.tile_pool(name="psx", bufs=2, space="PSUM") as psxp,
            tc.tile_pool(name="pss", bufs=3, space="PSUM") as pssp,
            tc.tile_pool(name="pszr", bufs=2, space="PSUM") as pszrp,
            tc.tile_pool(name="psh", bufs=1, space="PSUM") as pshp,
        ):
            # ---- constants ----
            # fcw first so pair 0 can start immediately; adjT is host-pre-tiled
            # to SBUF layout (contiguous 32KB per partition -> cheap DMA)
            fcw_sb = constp.tile([2 * F_IN, 2 * H1], fp8)
            nc.sync.dma_start(out=fcw_sb[:], in_=fcw_d[:])
            # adjT in fp8 against bf16 xh (mixed-dtype matmul, allowed for non-f32):
            # halves the startup-critical adjacency load to 2MB
            adjT_sb = constp.tile([128, KT, NS], fp8)
            # Fine 4-ktile chunks land in k-order so pair 0's S burst can
            # stream along as they arrive. Chunks 0-5 alternate
            # gpsimd/scalar here; chunks 6-7 are emitted AFTER the prologue
            # xt DMAs so they ride the then-free sync queue (a third lane)
            # without making pair 0's xh wait behind adjT.
            for q in range(6):
                eng = nc.gpsimd if q % 2 == 0 else nc.scalar
                eng.dma_start(
                    out=adjT_sb[:, q * 4 : (q + 1) * 4, :],
                    in_=adjT_d[:, q * 4 : (q + 1) * 4, :],
                )
            wzr_sb = constp.tile([128, 128], bf16)
            wh_sb = constp.tile([128, F_OUT], bf16)
            bz_sb = constp.tile([F_OUT, 1], f32)
            bzn_sb = constp.tile([F_OUT, 1], f32)
            br_sb = constp.tile([F_OUT, 1], f32)
            bh_sb = constp.tile([F_OUT, 1], f32)
            for dst, src in (
                (wzr_sb, wzr_d), (wh_sb, wh_d),
                (bz_sb, bz_d), (bzn_sb, bzn_d), (br_sb, br_d), (bh_sb, bh_d),
            ):
                nc.gpsimd.dma_start(out=dst[:], in_=src[:])

            # ---- state ----
            # Concat rhs tiles for the K=128 gate matmuls: rows 0-63 carry
            # S_t.T (refreshed per step, off-chain), rows 64-127 carry the
            # recurrent state: h.T in CzS_*, (h*r).T in ChS_*. h ping-pongs
            # between the two CzS buffers (the combine writes the other one).
            # stacked per-partition bias [bz; br] for the fused zr sigmoid
            bzr_sb = constp.tile([128, 1], f32)
            nc.scalar.copy(bzr_sb[0:64, :], bz_sb[:])
            nc.scalar.copy(bzr_sb[64:128, :], br_sb[:])

            CzS = []
            ChS = []
            for i in range(6):
                czsi = statep.tile([128, NS], bf16, tag=f"CzS{i}")
                chsi = statep.tile([128, NS], bf16, tag=f"ChS{i}")
                CzS.append(czsi)
                ChS.append(chsi)
            nc.vector.memset(CzS[0][:], 0.0)

            S_prev = None  # S-pair tile of the previous pair

            def emit_xh_groups(xt, xh, groups):
                # xh-pair matmuls: out[128 nodes, 64] = xT_slice.T @ fcw.
                # All operands at partition base 0 (base-64 matmul operands
                # crash the exec unit on TRN2). Steps t / t+1 select the free
                # axis of xt. 8 matmuls fill one PSUM bank laid out to match
                # xh's [node-tile, t|t+1] column order.
                for g in groups:
                    ps = psxp.tile([128, 512], mybir.dt.float32)
                    for j in range(4):
                        k = 4 * g + j
                        nc.tensor.matmul(
                            ps[:, j * 128 : (j + 1) * 128],
                            lhsT=xt[:, k * 128 : (k + 1) * 128],
                            rhs=fcw_sb[:],
                            start=True, stop=True,
                        )
                    nc.any.tensor_relu(
                        xh[:, 4 * g : 4 * (g + 1), :].rearrange("p a b -> p (a b)"),
                        ps[:],
                    )

            def emit_gru_front(step):
                # zr matmul + sigmoids + h*r / z*h products for one step.
                cur = CzS[step % 6]
                ch = ChS[step % 6]
                H = cur[64:128, :]

                ps_zr = pszrp.tile([128, NS], mybir.dt.float32, tag="ps_zr")
                nc.tensor.matmul(ps_zr[:], lhsT=wzr_sb[:], rhs=cur[:],
                                 start=True, stop=True)
                # ONE fused sigmoid computes BOTH gates (ACT cost scales with
                # free size only): z lands at rows 0:64, r at 64:128 aligned
                # with h. zc = 1-z via a partition-shifted tensor_scalar;
                # z*h recovered as h - zc*h.
                ZR = gwp.tile([128, NS], bf16, tag="ZR")
                nc.scalar.activation(ZR[:], ps_zr[:], AF.Sigmoid,
                                     bias=bzr_sb[:])
                ZC = gwp.tile([128, NS], bf16, tag="ZC")
                nc.vector.tensor_scalar(ZC[64:128, :], ZR[0:64, :], -1.0, 1.0,
                                        mybir.AluOpType.mult, mybir.AluOpType.add)
                nc.vector.tensor_mul(ch[64:128, :], H, ZR[64:128, :])
                A1p = gwp.tile([128, NS], bf16, tag="A1p")
                nc.vector.tensor_mul(A1p[64:128, :], ZC[64:128, :], H)
                A1 = gwp.tile([128, NS], bf16, tag="A1")
                nc.vector.tensor_sub(A1[64:128, :], H, A1p[64:128, :])
                return ZC, A1

            def emit_gru_back(step, ZC, A1, out=None):
                # h-candidate matmul + tanh + combine into the next buffer.
                ch = ChS[step % 6]
                nxt = CzS[(step + 1) % 6]
                ps_h = pshp.tile([F_OUT, NS], mybir.dt.float32)
                nc.tensor.matmul(ps_h[:], lhsT=wh_sb[:], rhs=ch[:],
                                 start=True, stop=True)
                HT = gwp.tile([128, NS], bf16, tag="HT")
                nc.scalar.activation(HT[64:128, :], ps_h[:], AF.Tanh,
                                     bias=bh_sb[:])
                # h_new = z*h + (1-z)*ht -> bottom half of the NEXT buffer
                # (or the caller's override, e.g. the f32 output tile)
                B1 = gwp.tile([128, NS], bf16, tag="B1")
                nc.vector.tensor_mul(B1[64:128, :], ZC[64:128, :], HT[64:128, :])
                dst = nxt[64:128, :] if out is None else out
                nc.vector.tensor_add(dst, A1[64:128, :], B1[64:128, :])

            # ---- main loop, software-pipelined: gates of pair p-1 are
            # emitted between the xh/A matmul bursts of pair p so the
            # sequential GRU chain hides under parallel PE work. ----
            def emit_xt(p):
                xt = xtp.tile([2 * F_IN, N], fp8)
                # alternate DMA queues (sync/gpsimd: keeps the ACT queue --
                # which carries the chain sigmoids -- free of DMA triggers)
                (nc.sync if p % 2 == 0 else nc.gpsimd).dma_start(
                    out=xt[:], in_=xT_d[p]
                )
                return xt

            # xh runs TWO pairs ahead of its S burst: at startup the PE
            # fills xh for pairs 0-1 while the 4MB adjT load streams in, so
            # pair 0's S burst doesn't stall on adjT mid-contraction. Both
            # prologue xt DMAs go on sync -- the gpsimd queue is busy with
            # adjT chunks, and xt(1) queued behind 2MB of adjT stalls the
            # whole PE pipeline at xh(1).
            xh_store = {}
            for q in (0, 1):
                xt = xtp.tile([2 * F_IN, N], fp8, tag=f"xt_pro{q}")
                nc.sync.dma_start(out=xt[:], in_=xT_d[q])
                if q == 1:
                    # deferred adjT chunks 6-7 on the now-free sync queue
                    for qq in (6, 7):
                        nc.sync.dma_start(
                            out=adjT_sb[:, qq * 4 : (qq + 1) * 4, :],
                            in_=adjT_d[:, qq * 4 : (qq + 1) * 4, :],
                        )
                xh_q = xhp.tile([128, KT, 128], bf16, tag="xh")
                emit_xh_groups(xt, xh_q, range(0, 8))
                xh_store[q] = xh_q

            for p in range(PAIRS):
                xh = xh_store.pop(p)
                nxt = None
                if p + 2 < PAIRS:
                    xt = emit_xt(p + 2)
                    nxt = xhp.tile([128, KT, 128], bf16, tag="xh")
                    xh_store[p + 2] = nxt

                def xh_grp(groups):
                    if nxt is not None:
                        emit_xh_groups(xt, nxt, groups)

                # the GRU steps of pair p-2 are interleaved between the
                # xh matmul groups (of pair p+2): every chain wait (ACT
                # sigma / DVE mul) is covered by queued PE work, and the
                # 2-pair gate lag keeps the final S bursts off the chain
                if p >= 2:
                    fr0 = emit_gru_front(2 * p - 4)
                xh_grp(range(0, 2))
                if p >= 2:
                    emit_gru_back(2 * p - 4, *fr0)
                xh_grp(range(2, 4))
                if p >= 2:
                    fr1 = emit_gru_front(2 * p - 3)
                xh_grp(range(4, 6))
                if p >= 2:
                    emit_gru_back(2 * p - 3, *fr1)
                xh_grp(range(6, 8))

                # S-pair matmul: psS[2*64 feat, 512 my-nodes] accumulated
                # over 32 node K-tiles. bf16 (not fp8 DoubleRow): the power
                # limiter throttles DoubleRow's 2x MAC density back to ~1x
                # anyway, and bf16 halves the PE's peak draw during the burst
                # so the concurrently running chain ops throttle less.
                psS = pssp.tile([128, NS], mybir.dt.float32)
                for k in range(KT):
                    nc.tensor.matmul(
                        psS[:],
                        lhsT=xh[:, k, :],
                        rhs=adjT_sb[:, k, :],
                        start=(k == 0), stop=(k == KT - 1),
                    )
                # refresh concat tops for this pair's two steps; the 6-way
                # rotation means these buffers were last read three pairs
                # ago, so the copies run fully off the sequential gate chain
                s0, s1 = (2 * p) % 6, (2 * p + 1) % 6
                nc.scalar.copy(CzS[s0][0:64, :], psS[0:64, :])
                nc.vector.tensor_copy(ChS[s0][0:64, :], psS[0:64, :])
                nc.scalar.copy(CzS[s1][0:64, :], psS[64:128, :])
                nc.vector.tensor_copy(ChS[s1][0:64, :], psS[64:128, :])

            # drain: gates for the last two pairs; the final combine writes
            # the f32 output tile directly (rows 64:128 keep the DVE add
            # partition-aligned with its bf16 inputs), skipping a copy
            Hout = statep.tile([128, NS], f32)
            for s in range(2 * PAIRS - 4, 2 * PAIRS):
                fr = emit_gru_front(s)
                emit_gru_back(s, *fr,
                              out=Hout[64:128, :] if s == 2 * PAIRS - 1
                              else None)

            nc.sync.dma_start(out=out_d[:], in_=Hout[64:128, :])

    nc.compile()
    return nc


def _prep_inputs(x, adj, fc_w, Wz, Wr, Wh, Lz, Lr, Lh, bz, br, bh):
    bf16 = ml_dtypes.bfloat16
    fp8 = ml_dtypes.float8_e4m3fn
    f32 = np.float32

    # x [T, N, F] -> [PAIRS, (step, F), N]: both steps of a pair stacked on
    # the partition axis, matched by a block-diagonal fcw
    xT = np.ascontiguousarray(
        x.reshape(PAIRS, 2, N, F_IN).transpose(0, 1, 3, 2).reshape(
            PAIRS, 2 * F_IN, N
        )
    ).astype(fp8)
    fcw = np.zeros((2 * F_IN, 2 * H1), np.float32)
    fcw[0:F_IN, 0:H1] = fc_w
    fcw[F_IN:, H1:] = fc_w
    fcw = fcw.astype(fp8)

    # adj entries are U[0, 1/N] -- far below fp8e4m3's subnormal floor.
    # Pre-scale by N=2^12 (exact) so they quantize as U[0,1]; the inverse
    # 2^-12 folds into the gate-weight rows that consume S (also exact).
    ADJ_SCALE = 4096.0

    def fold(W, L):
        return (
            (W.astype(np.float64) @ L[:F_OUT].astype(np.float64)) / ADJ_SCALE
        ).astype(bf16)

    mz, mr, mh = fold(Wz, Lz), fold(Wr, Lr), fold(Wh, Lh)
    mzr = np.concatenate([mz, mr], axis=1)  # [64, 128]: z cols | r cols
    lzr = np.concatenate(
        [Lz[F_OUT:].astype(bf16), Lr[F_OUT:].astype(bf16)], axis=1
    )
    # stacked [K=128] weights: rows 0-63 hit S_t, rows 64-127 hit h / (h*r)
    wzr = np.concatenate([mzr, lzr], axis=0)  # [128, 128]
    wh = np.concatenate(
        [mh, Lh[F_OUT:].astype(bf16)], axis=0
    )  # [128, 64]
    shared = {
        "xT": xT, "fcw": fcw, "wzr": wzr, "wh": wh,
        "bz": bz.reshape(F_OUT, 1).astype(f32),
        "bzn": (-bz).reshape(F_OUT, 1).astype(f32),
        "br": br.reshape(F_OUT, 1).astype(f32),
        "bh": bh.reshape(F_OUT, 1).astype(f32),
    }
    in_maps = []
    for c in range(NCORES):
        m = dict(shared)
        at = adj[c * NS : (c + 1) * NS, :].T * ADJ_SCALE  # [N, NS]
        m["adjT"] = np.ascontiguousarray(
            at.reshape(KT, 128, NS).transpose(1, 0, 2)
        ).astype(ml_dtypes.float8_e4m3fn)
        in_maps.append(m)
    return in_maps


def kernel(x, adj, fc_w, fc_b, Wz, Wr, Wh, Lz, Lr, Lh, bz, br, bh):
    x = np.asarray(x, np.float32)
    adj = np.asarray(adj, np.float32)
    args = [np.asarray(a, np.float32) for a in (fc_w, Wz, Wr, Wh, Lz, Lr, Lh, bz, br, bh)]
    fc_b = np.asarray(fc_b, np.float32)
    if np.any(fc_b != 0.0):
        # fc_b can't fold into the per-partition activation bias (it varies
        # along the free dim); the reference always passes zeros. Pure-numpy
        # fallback keeps kernel() correct for arbitrary inputs.
        return _numpy_ref(x, adj, args[0], fc_b, *args[1:])

    from concourse.bass_utils import run_bass_kernel_spmd

    if "nc" not in _cache:
        _cache["nc"] = _build()
    nc = _cache["nc"]

    in_maps = _prep_inputs(x, adj, *args)
    trace = bool(int(os.environ.get("BASS_KERNEL_TRACE", "0")))
    kwargs = {}
    if trace:
        _install_trace_shim()
        tmpdir = os.environ.get("BASS_KERNEL_TRACE_DIR")
        if tmpdir:
            os.makedirs(tmpdir, exist_ok=True)
            kwargs["tmpdir"] = tmpdir
    res = run_bass_kernel_spmd(
        nc, in_maps, core_ids=list(range(NCORES)), trace=trace, **kwargs
    )
    _cache["last_result"] = res

    out = np.empty((1, N, F_OUT), np.float32)
    for c in range(NCORES):
        out[0, c * NS : (c + 1) * NS, :] = res.results[c]["out"].T
    return out


def _install_trace_shim():
    """Register the NTFF profile hook (this image's antenv lacks axon_hooks)
    and stub out the artifact upload so profiling works offline."""
    import types

    try:
        from antenv import axon_hooks  # noqa: F401
        return
    except ImportError:
        pass
    sys.path.insert(0, "/root/.axon_site")
    from trn_agent_boot.trn_boot import _ntff_profile_via_ctypes

    hook = _ntff_profile_via_ctypes("/opt/axon/libaxon_pjrt.so")
    m = types.ModuleType("antenv.axon_hooks")
    m.get_axon_ntff_profile_hook = lambda: hook
    m.set_axon_ntff_profile_hook = lambda h: None
    sys.modules["antenv.axon_hooks"] = m
    import antenv

    antenv.axon_hooks = m
    from concourse import bass_utils as _bu

    _bu.upload_artifacts = lambda tmpdir: tmpdir


def _numpy_ref(x, adj, fc_w, fc_b, Wz, Wr, Wh, Lz, Lr, Lh, bz, br, bh):
    def sigmoid(v):
        return 1.0 / (1.0 + np.exp(-v))

    xh = np.maximum(x @ fc_w + fc_b, 0.0)
    h = np.zeros((N, F_OUT), np.float32)
    for t in range(T):
        s = adj @ xh[t]
        az, ar, ah = s @ Wz, s @ Wr, s @ Wh
        z = sigmoid(np.concatenate([az, h], -1) @ Lz + bz)
        r = sigmoid(np.concatenate([ar, h], -1) @ Lr + br)
        ht = np.tanh(np.concatenate([ah, h * r], -1) @ Lh + bh)
        h = z * h + (1.0 - z) * ht
    return h[None].astype(np.float32)



# revision 73
# speedup vs baseline: 1.0206x; 1.0206x over previous
"""TGCN (dense-graph GRU) Trainium2 kernel, 8-core SPMD, no collectives.

Math (per reference):
  xh_t = relu(x_t @ fc_w + fc_b)                    [N, H]
  S_t  = adj @ xh_t                                 (assoc: adj@(xh@W) = (adj@xh)@W)
  z_t  = sigmoid(S_t @ Mz + h @ Lz_bot + bz)        Mz = Wz @ Lz_top (host-folded)
  r_t  = sigmoid(S_t @ Mr + h @ Lr_bot + br)
  ht_t = tanh   (S_t @ Mh + (h*r) @ Lh_bot + bh)
  h    = z*h + (1-z)*ht = ht + z*(h - ht)

Sharding: row-partition adj across 8 cores (512 nodes each). The GRU cell is
row-local, so each core runs the whole time loop on its shard independently.
x is replicated (each core redundantly computes xh for all nodes — cheaper
than an all-gather through DRAM bounce buffers).

Layout: everything on-chip is feature-major ("transposed"): S_t.T, z.T, h.T
are [64 feat, 512 nodes]. This makes every matmul operand natural-layout
(weights [K, M] as stored, x host-transposed to [T, F, N]) — zero on-chip
transposes. Time steps are processed in pairs so the big adj matmul has
M=128 (full PE array): lhsT = [xh_t | xh_t+1] tiles, rhs = adjT tiles.

Each GRU gate is ONE K=128 matmul: stationary [M*; L*_bot] stacked on the
contraction dim, rhs a [S_t.T; h.T] concat tile whose bottom half IS the
recurrent state (the combine writes h directly into the next concat buffer,
rotating over 6 buffers so gates can lag their pair's S matmul by 2 pairs,
keeping the final S bursts off the chain tail). ACT's ability to
read/write at shifted partition bases glues the [0:64]/[64:128] halves
together; matmul operands at base 64 crash TRN2, so all matmul rhs/lhsT
tiles start at partition 0 or are full 128-partition.

Dtypes: x/fcw fp8e4 (halves the streamed-x DMA; the xh matmul computes
both steps of a pair in one K=128 matmul against a block-diagonal fcw with
steps stacked on the partition axis). adj/xh/S and all other matmuls bf16,
adj host-prescaled by N=2^12 with the inverse folded into the gate weights
(both exact powers of two). bf16 -- NOT fp8 DoubleRow -- for the big S
matmul: the chip's power limiter throttles DoubleRow's 2x MAC density back
to ~1x whenever DVE/ACT run concurrently (measured: 42% -> 8% throttle
residency switching DR -> bf16), so bf16 streams just as fast and leaves
headroom for the chain's ACT/DVE ops. Keep the S burst contiguous per
pair; interleaving it with other matmuls measurably degrades the II. An
AllGather-sharded-xh variant was tried and is SLOWER (collective ring runs
~85 GB/s effective, ~24us per 2MB gather -- communication exceeds the
saved compute). h state bf16; PSUM f32.
Scheduling: xh computation runs TWO pairs ahead of its S burst (a
prologue fills xh for pairs 0-1 while the 4MB adjT load streams in on all
three DMA-capable queues in fine k-ordered chunks), gates lag their pair
by 2, and the final GRU combine writes the f32 output tile directly.
Measured: ~287us HW exec (baseline 362us), rel err ~6.5e-3 (tol 2e-2).
Beware device-level run-to-run variance: identical binaries occasionally
measure ~340us (same per-core throttle counters; shared-device state).
"""

import os
import sys

sys.path.insert(0, "/opt/trn_rl_repo")

import numpy as np
import ml_dtypes

T, N, F_IN, H1, F_OUT = 48, 4096, 64, 64, 64
NCORES = 8
NS = N // NCORES          # nodes per core = 512
PAIRS = T // 2            # 24
KT = N // 128             # 32 contraction tiles for the adj matmul

_cache = {}


def _build():
    import concourse.bass as bass
    import concourse.mybir as mybir
    import concourse.tile as tile
    from concourse import bacc

    f32 = mybir.dt.float32
    f32r = mybir.dt.float32r
    bf16 = mybir.dt.bfloat16
    fp8 = mybir.dt.float8e4
    DR = mybir.MatmulPerfMode.DoubleRow
    AF = mybir.ActivationFunctionType

    nc = bacc.Bacc(
        "TRN2",
        target_bir_lowering=False,
        debug=False,
        enable_asserts=False,
        num_devices=NCORES,
    )

    # DRAM parameters (per-core shapes)
    adjT_d = nc.dram_tensor("adjT", [128, KT, NS], fp8, kind="ExternalInput").ap()
    # steps stacked on partitions: one K=128 matmul per node tile computes
    # BOTH steps' xh against a block-diagonal fcw
    xT_d = nc.dram_tensor("xT", [PAIRS, 2 * F_IN, N], fp8, kind="ExternalInput").ap()
    fcw_d = nc.dram_tensor("fcw", [2 * F_IN, 2 * H1], fp8, kind="ExternalInput").ap()
    wzr_d = nc.dram_tensor("wzr", [128, 128], bf16, kind="ExternalInput").ap()
    wh_d = nc.dram_tensor("wh", [128, F_OUT], bf16, kind="ExternalInput").ap()
    bz_d = nc.dram_tensor("bz", [F_OUT, 1], f32, kind="ExternalInput").ap()
    bzn_d = nc.dram_tensor("bzn", [F_OUT, 1], f32, kind="ExternalInput").ap()
    br_d = nc.dram_tensor("br", [F_OUT, 1], f32, kind="ExternalInput").ap()
    bh_d = nc.dram_tensor("bh", [F_OUT, 1], f32, kind="ExternalInput").ap()
    out_d = nc.dram_tensor("out", [F_OUT, NS], f32, kind="ExternalOutput").ap()

    with tile.TileContext(nc) as tc:
        with (
            tc.tile_pool(name="const", bufs=1) as constp,
            tc.tile_pool(name="state", bufs=1) as statep,
            tc.tile_pool(name="xt", bufs=2) as xtp,
            tc.tile_pool(name="xh", bufs=3) as xhp,
            tc.tile_pool(name="gw", bufs=3) as gwp,
            tc.tile_pool(name="psx", bufs=2, space="PSUM") as psxp,
            tc.tile_pool(name="pss", bufs=3, space="PSUM") as pssp,
            tc.tile_pool(name="pszr", bufs=2, space="PSUM") as pszrp,
            tc.tile_pool(name="psh", bufs=1, space="PSUM") as pshp,
        ):
            # ---- constants ----
            # fcw first so pair 0 can start immediately; adjT is host-pre-tiled
            # to SBUF layout (contiguous 32KB per partition -> cheap DMA)
            fcw_sb = constp.tile([2 * F_IN, 2 * H1], fp8)
            nc.sync.dma_start(out=fcw_sb[:], in_=fcw_d[:])
            # adjT in fp8 against bf16 xh (mixed-dtype matmul, allowed for non-f32):
            # halves the startup-critical adjacency load to 2MB
            adjT_sb = constp.tile([128, KT, NS], fp8)
            # Fine 4-ktile chunks land in k-order so pair 0's S burst can
            # stream along as they arrive. Chunks 0-5 alternate
            # gpsimd/scalar here; chunks 6-7 are emitted AFTER the prologue
            # xt DMAs so they ride the then-free sync queue (a third lane)
            # without making pair 0's xh wait behind adjT.
            for q in range(6):
                eng = nc.gpsimd if q % 2 == 0 else nc.scalar
                eng.dma_start(
                    out=adjT_sb[:, q * 4 : (q + 1) * 4, :],
                    in_=adjT_d[:, q * 4 : (q + 1) * 4, :],
                )
            wzr_sb = constp.tile([128, 128], bf16)
            wh_sb = constp.tile([128, F_OUT], bf16)
            bz_sb = constp.tile([F_OUT, 1], f32)
            bzn_sb = constp.tile([F_OUT, 1], f32)
            br_sb = constp.tile([F_OUT, 1], f32)
            bh_sb = constp.tile([F_OUT, 1], f32)
            for dst, src in (
                (wzr_sb, wzr_d), (wh_sb, wh_d),
                (bz_sb, bz_d), (bzn_sb, bzn_d), (br_sb, br_d), (bh_sb, bh_d),
            ):
                nc.gpsimd.dma_start(out=dst[:], in_=src[:])

            # ---- state ----
            # Concat rhs tiles for the K=128 gate matmuls: rows 0-63 carry
            # S_t.T (refreshed per step, off-chain), rows 64-127 carry the
            # recurrent state: h.T in CzS_*, (h*r).T in ChS_*. h ping-pongs
            # between the two CzS buffers (the combine writes the other one).
            CzS = []
            ChS = []
            for i in range(6):
                czsi = statep.tile([128, NS], bf16, tag=f"CzS{i}")
                chsi = statep.tile([128, NS], bf16, tag=f"ChS{i}")
                CzS.append(czsi)
                ChS.append(chsi)
            nc.vector.memset(CzS[0][:], 0.0)

            S_prev = None  # S-pair tile of the previous pair

            def emit_xh_groups(xt, xh, groups):
                # xh-pair matmuls: out[128 nodes, 64] = xT_slice.T @ fcw.
                # All operands at partition base 0 (base-64 matmul operands
                # crash the exec unit on TRN2). Steps t / t+1 select the free
                # axis of xt. 8 matmuls fill one PSUM bank laid out to match
                # xh's [node-tile, t|t+1] column order.
                for g in groups:
                    ps = psxp.tile([128, 512], mybir.dt.float32)
                    for j in range(4):
                        k = 4 * g + j
                        nc.tensor.matmul(
                            ps[:, j * 128 : (j + 1) * 128],
                            lhsT=xt[:, k * 128 : (k + 1) * 128],
                            rhs=fcw_sb[:],
                            start=True, stop=True,
                        )
                    nc.any.tensor_relu(
                        xh[:, 4 * g : 4 * (g + 1), :].rearrange("p a b -> p (a b)"),
                        ps[:],
                    )

            def emit_gru_front(step):
                # zr matmul + sigmoids + h*r / z*h products for one step.
                cur = CzS[step % 6]
                ch = ChS[step % 6]
                H = cur[64:128, :]

                ps_zr = pszrp.tile([128, NS], mybir.dt.float32, tag="ps_zr")
                nc.tensor.matmul(ps_zr[:], lhsT=wzr_sb[:], rhs=cur[:],
                                 start=True, stop=True)
                # r first: it gates the h-candidate matmul (critical chain);
                # z / (1-z) / z*h all run off-chain in parallel
                R = gwp.tile([128, NS], bf16, tag="R")
                nc.scalar.activation(R[64:128, :], ps_zr[64:128, :],
                                     AF.Sigmoid, bias=br_sb[:])
                Z = gwp.tile([128, NS], bf16, tag="Z")
                nc.scalar.activation(Z[64:128, :], ps_zr[0:64, :],
                                     AF.Sigmoid, bias=bz_sb[:])
                ZC = gwp.tile([128, NS], bf16, tag="ZC")
                nc.vector.tensor_scalar(ZC[64:128, :], Z[64:128, :], -1.0, 1.0,
                                        mybir.AluOpType.mult, mybir.AluOpType.add)
                nc.vector.tensor_mul(ch[64:128, :], H, R[64:128, :])
                A1 = gwp.tile([128, NS], bf16, tag="A1")
                nc.vector.tensor_mul(A1[64:128, :], Z[64:128, :], H)
                return ZC, A1

            def emit_gru_back(step, ZC, A1, out=None):
                # h-candidate matmul + tanh + combine into the next buffer.
                ch = ChS[step % 6]
                nxt = CzS[(step + 1) % 6]
                ps_h = pshp.tile([F_OUT, NS], mybir.dt.float32)
                nc.tensor.matmul(ps_h[:], lhsT=wh_sb[:], rhs=ch[:],
                                 start=True, stop=True)
                HT = gwp.tile([128, NS], bf16, tag="HT")
                nc.scalar.activation(HT[64:128, :], ps_h[:], AF.Tanh,
                                     bias=bh_sb[:])
                # h_new = z*h + (1-z)*ht -> bottom half of the NEXT buffer
                # (or the caller's override, e.g. the f32 output tile)
                B1 = gwp.tile([128, NS], bf16, tag="B1")
                nc.vector.tensor_mul(B1[64:128, :], ZC[64:128, :], HT[64:128, :])
                dst = nxt[64:128, :] if out is None else out
                nc.vector.tensor_add(dst, A1[64:128, :], B1[64:128, :])

            # ---- main loop, software-pipelined: gates of pair p-1 are
            # emitted between the xh/A matmul bursts of pair p so the
            # sequential GRU chain hides under parallel PE work. ----
            def emit_xt(p):
                xt = xtp.tile([2 * F_IN, N], fp8)
                # alternate DMA queues (sync/gpsimd: keeps the ACT queue --
                # which carries the chain sigmoids -- free of DMA triggers)
                (nc.sync if p % 2 == 0 else nc.gpsimd).dma_start(
                    out=xt[:], in_=xT_d[p]
                )
                return xt

            # xh runs TWO pairs ahead of its S burst: at startup the PE
            # fills xh for pairs 0-1 while the 4MB adjT load streams in, so
            # pair 0's S burst doesn't stall on adjT mid-contraction. Both
            # prologue xt DMAs go on sync -- the gpsimd queue is busy with
            # adjT chunks, and xt(1) queued behind 2MB of adjT stalls the
            # whole PE pipeline at xh(1).
            xh_store = {}
            for q in (0, 1):
                xt = xtp.tile([2 * F_IN, N], fp8, tag=f"xt_pro{q}")
                nc.sync.dma_start(out=xt[:], in_=xT_d[q])
                if q == 1:
                    # deferred adjT chunks 6-7 on the now-free sync queue
                    for qq in (6, 7):
                        nc.sync.dma_start(
                            out=adjT_sb[:, qq * 4 : (qq + 1) * 4, :],
                            in_=adjT_d[:, qq * 4 : (qq + 1) * 4, :],
                        )
                xh_q = xhp.tile([128, KT, 128], bf16, tag="xh")
                emit_xh_groups(xt, xh_q, range(0, 8))
                xh_store[q] = xh_q

            for p in range(PAIRS):
                xh = xh_store.pop(p)
                nxt = None
                if p + 2 < PAIRS:
                    xt = emit_xt(p + 2)
                    nxt = xhp.tile([128, KT, 128], bf16, tag="xh")
                    xh_store[p + 2] = nxt

                def xh_grp(groups):
                    if nxt is not None:
                        emit_xh_groups(xt, nxt, groups)

                # the GRU steps of pair p-2 are interleaved between the
                # xh matmul groups (of pair p+2): every chain wait (ACT
                # sigma / DVE mul) is covered by queued PE work, and the
                # 2-pair gate lag keeps the final S bursts off the chain
                if p >= 2:
                    fr0 = emit_gru_front(2 * p - 4)
                xh_grp(range(0, 2))
                if p >= 2:
                    emit_gru_back(2 * p - 4, *fr0)
                xh_grp(range(2, 4))
                if p >= 2:
                    fr1 = emit_gru_front(2 * p - 3)
                xh_grp(range(4, 6))
                if p >= 2:
                    emit_gru_back(2 * p - 3, *fr1)
                xh_grp(range(6, 8))

                # S-pair matmul: psS[2*64 feat, 512 my-nodes] accumulated
                # over 32 node K-tiles. bf16 (not fp8 DoubleRow): the power
                # limiter throttles DoubleRow's 2x MAC density back to ~1x
                # anyway, and bf16 halves the PE's peak draw during the burst
                # so the concurrently running chain ops throttle less.
                psS = pssp.tile([128, NS], mybir.dt.float32)
                for k in range(KT):
                    nc.tensor.matmul(
                        psS[:],
                        lhsT=xh[:, k, :],
                        rhs=adjT_sb[:, k, :],
                        start=(k == 0), stop=(k == KT - 1),
                    )
                # refresh concat tops for this pair's two steps; the 6-way
                # rotation means these buffers were last read three pairs
                # ago, so the copies run fully off the sequential gate chain
                s0, s1 = (2 * p) % 6, (2 * p + 1) % 6
                nc.scalar.copy(CzS[s0][0:64, :], psS[0:64, :])
                nc.vector.tensor_copy(ChS[s0][0:64, :], psS[0:64, :])
                nc.scalar.copy(CzS[s1][0:64, :], psS[64:128, :])
                nc.vector.tensor_copy(ChS[s1][0:64, :], psS[64:128, :])

            # drain: gates for the last two pairs; the final combine writes
            # the f32 output tile directly (rows 64:128 keep the DVE add
            # partition-aligned with its bf16 inputs), skipping a copy
            Hout = statep.tile([128, NS], f32)
            for s in range(2 * PAIRS - 4, 2 * PAIRS):
                fr = emit_gru_front(s)
                emit_gru_back(s, *fr,
                              out=Hout[64:128, :] if s == 2 * PAIRS - 1
                              else None)

            nc.sync.dma_start(out=out_d[:], in_=Hout[64:128, :])

    nc.compile()
    return nc


def _prep_inputs(x, adj, fc_w, Wz, Wr, Wh, Lz, Lr, Lh, bz, br, bh):
    bf16 = ml_dtypes.bfloat16
    fp8 = ml_dtypes.float8_e4m3fn
    f32 = np.float32

    # x [T, N, F] -> [PAIRS, (step, F), N]: both steps of a pair stacked on
    # the partition axis, matched by a block-diagonal fcw
    xT = np.ascontiguousarray(
        x.reshape(PAIRS, 2, N, F_IN).transpose(0, 1, 3, 2).reshape(
            PAIRS, 2 * F_IN, N
        )
    ).astype(fp8)
    fcw = np.zeros((2 * F_IN, 2 * H1), np.float32)
    fcw[0:F_IN, 0:H1] = fc_w
    fcw[F_IN:, H1:] = fc_w
    fcw = fcw.astype(fp8)

    # adj entries are U[0, 1/N] -- far below fp8e4m3's subnormal floor.
    # Pre-scale by N=2^12 (exact) so they quantize as U[0,1]; the inverse
    # 2^-12 folds into the gate-weight rows that consume S (also exact).
    ADJ_SCALE = 4096.0

    def fold(W, L):
        return (
            (W.astype(np.float64) @ L[:F_OUT].astype(np.float64)) / ADJ_SCALE
        ).astype(bf16)

    mz, mr, mh = fold(Wz, Lz), fold(Wr, Lr), fold(Wh, Lh)
    mzr = np.concatenate([mz, mr], axis=1)  # [64, 128]: z cols | r cols
    lzr = np.concatenate(
        [Lz[F_OUT:].astype(bf16), Lr[F_OUT:].astype(bf16)], axis=1
    )
    # stacked [K=128] weights: rows 0-63 hit S_t, rows 64-127 hit h / (h*r)
    wzr = np.concatenate([mzr, lzr], axis=0)  # [128, 128]
    wh = np.concatenate(
        [mh, Lh[F_OUT:].astype(bf16)], axis=0
    )  # [128, 64]
    shared = {
        "xT": xT, "fcw": fcw, "wzr": wzr, "wh": wh,
        "bz": bz.reshape(F_OUT, 1).astype(f32),
        "bzn": (-bz).reshape(F_OUT, 1).astype(f32),
        "br": br.reshape(F_OUT, 1).astype(f32),
        "bh": bh.reshape(F_OUT, 1).astype(f32),
    }
    in_maps = []
    for c in range(NCORES):
        m = dict(shared)
        at = adj[c * NS : (c + 1) * NS, :].T * ADJ_SCALE  # [N, NS]
        m["adjT"] = np.ascontiguousarray(
            at.reshape(KT, 128, NS).transpose(1, 0, 2)
        ).astype(ml_dtypes.float8_e4m3fn)
        in_maps.append(m)
    return in_maps


def kernel(x, adj, fc_w, fc_b, Wz, Wr, Wh, Lz, Lr, Lh, bz, br, bh):
    x = np.asarray(x, np.float32)
    adj = np.asarray(adj, np.float32)
    args = [np.asarray(a, np.float32) for a in (fc_w, Wz, Wr, Wh, Lz, Lr, Lh, bz, br, bh)]
    fc_b = np.asarray(fc_b, np.float32)
    if np.any(fc_b != 0.0):
        # fc_b can't fold into the per-partition activation bias (it varies
        # along the free dim); the reference always passes zeros. Pure-numpy
        # fallback keeps kernel() correct for arbitrary inputs.
        return _numpy_ref(x, adj, args[0], fc_b, *args[1:])

    from concourse.bass_utils import run_bass_kernel_spmd

    if "nc" not in _cache:
        _cache["nc"] = _build()
    nc = _cache["nc"]

    in_maps = _prep_inputs(x, adj, *args)
    trace = bool(int(os.environ.get("BASS_KERNEL_TRACE", "0")))
    kwargs = {}
    if trace:
        _install_trace_shim()
        tmpdir = os.environ.get("BASS_KERNEL_TRACE_DIR")
        if tmpdir:
            os.makedirs(tmpdir, exist_ok=True)
            kwargs["tmpdir"] = tmpdir
    res = run_bass_kernel_spmd(
        nc, in_maps, core_ids=list(range(NCORES)), trace=trace, **kwargs
    )
    _cache["last_result"] = res

    out = np.empty((1, N, F_OUT), np.float32)
    for c in range(NCORES):
        out[0, c * NS : (c + 1) * NS, :] = res.results[c]["out"].T
    return out


def _install_trace_shim():
    """Register the NTFF profile hook (this image's antenv lacks axon_hooks)
    and stub out the artifact upload so profiling works offline."""
    import types

    try:
        from antenv import axon_hooks  # noqa: F401
        return
    except ImportError:
        pass
    sys.path.insert(0, "/root/.axon_site")
    from trn_agent_boot.trn_boot import _ntff_profile_via_ctypes

    hook = _ntff_profile_via_ctypes("/opt/axon/libaxon_pjrt.so")
    m = types.ModuleType("antenv.axon_hooks")
    m.get_axon_ntff_profile_hook = lambda: hook
    m.set_axon_ntff_profile_hook = lambda h: None
    sys.modules["antenv.axon_hooks"] = m
    import antenv

    antenv.axon_hooks = m
    from concourse import bass_utils as _bu

    _bu.upload_artifacts = lambda tmpdir: tmpdir


def _numpy_ref(x, adj, fc_w, fc_b, Wz, Wr, Wh, Lz, Lr, Lh, bz, br, bh):
    def sigmoid(v):
        return 1.0 / (1.0 + np.exp(-v))

    xh = np.maximum(x @ fc_w + fc_b, 0.0)
    h = np.zeros((N, F_OUT), np.float32)
    for t in range(T):
        s = adj @ xh[t]
        az, ar, ah = s @ Wz, s @ Wr, s @ Wh
        z = sigmoid(np.concatenate([az, h], -1) @ Lz + bz)
        r = sigmoid(np.concatenate([ar, h], -1) @ Lr + br)
        ht = np.tanh(np.concatenate([ah, h * r], -1) @ Lh + bh)
        h = z * h + (1.0 - z) * ht
    return h[None].astype(np.float32)

